# revision 1
# baseline (speedup 1.0000x reference)
"""Causal GQA self-attention (dense_transformer) on 8 trn2 NeuronCores.

Sharding: core c -> (batch b = c//4, kv-group g = c%4).  Each core computes
the 4 query heads of its kv group against its batch element, plus the
partial output projection for those heads; the host sums the 4 partial
projections per batch (the all-reduce of the tensor-parallel proj).

On-device layout is fully transposed ("feature on partitions"):
  xT [D, S], qhat/khat [head_dim, S], scoresT [k_pos, q_pos], yT [head_dim, S],
  outT [D_out, S].  This makes every matmul contraction land on the partition
  dim with no on-device transposes of activations (RoPE's half-swap is done
  with a permutation matmul, v is transposed head_dim<->seq via the PE).
Softmax is computed without the max subtraction: q/k are RMS-normalized so
|scores| <= gain*sqrt(head_dim) and exp cannot overflow in fp32.
All matmuls run in float32r (~1e-4 rms rounding, full PE rate at N>=256).
"""

import numpy as np

import concourse.bacc as bacc
import concourse.mybir as mybir
import concourse.tile as tile
from concourse.bass_utils import run_bass_kernel_spmd

F32 = mybir.dt.float32
F32R = mybir.dt.float32r
BF16 = mybir.dt.bfloat16
import os as _os


def _dt_env(name, default="bf16"):
    return BF16 if _os.environ.get(name, default) == "bf16" else F32R


QKV_DT = _dt_env("KERNEL_QKV_DT")   # xs, wq/wk/wv, raw, sq, onsr, vraw/vtp/idn
SC_DT = _dt_env("KERNEL_SC_DT")     # qhat, khat, swp (scores matmul + rope swap)
AV_DT = _dt_env("KERNEL_AV_DT")     # et, vnat, ons, msk (attn-weights path)
PR_DT = _dt_env("KERNEL_PR_DT")     # yn, wp (output projection)
AF = mybir.ActivationFunctionType
ALU = mybir.AluOpType

B = 2
S = 2048
D = 2048
N_HEADS = 16
N_KV = 4
HD = 128
G = N_HEADS // N_KV  # 4 query heads per core
ROPE_BASE = 10000.0
RMS_EPS = 1.1920928955078125e-07
NCH = D // 128       # 16 contraction chunks for the projections
ST = 512             # s-tile width (projection phase)
QT = 512             # q-tile width (attention phase)
KC = 128             # k chunk (scoresT partition block)
GRP = 1              # k chunks per exp group


def _build_program(s_len=S):
    nst = s_len // ST
    nqt = s_len // QT
    nc = bacc.Bacc("TRN2", target_bir_lowering=False, debug=False, num_devices=8)

    xT = nc.dram_tensor("xT", [D, s_len], QKV_DT, kind="ExternalInput")
    wq = nc.dram_tensor("wq", [D, G * HD], QKV_DT, kind="ExternalInput")
    wk = nc.dram_tensor("wk", [D, HD], QKV_DT, kind="ExternalInput")
    wv = nc.dram_tensor("wv", [D, HD], QKV_DT, kind="ExternalInput")
    wp = nc.dram_tensor("wp", [G * HD, D], PR_DT, kind="ExternalInput")
    cos2 = nc.dram_tensor("cos2", [128, s_len], F32R, kind="ExternalInput")
    sin2 = nc.dram_tensor("sin2", [128, s_len], F32R, kind="ExternalInput")
    gains = nc.dram_tensor("gains", [128, G], F32, kind="ExternalInput")
    swp = nc.dram_tensor("swp", [128, 128], SC_DT, kind="ExternalInput")
    idn = nc.dram_tensor("idn", [128, 128], QKV_DT, kind="ExternalInput")
    ons = nc.dram_tensor("ons", [128, 128], AV_DT, kind="ExternalInput")
    msk = nc.dram_tensor("msk", [128, 4 * QT], AV_DT, kind="ExternalInput")
    out = nc.dram_tensor("out", [D, s_len], F32, kind="ExternalOutput")

    with tile.TileContext(nc) as tc:
        with tc.tile_pool(name="persist", bufs=1) as pp, \
             tc.tile_pool(name="const", bufs=1) as cp:
            qhat = pp.tile([128, G, s_len], SC_DT)
            khat = pp.tile([128, s_len], SC_DT)
            vnat = pp.tile([128, s_len // 128, HD], AV_DT)
            yn = pp.tile([128, G, s_len], PR_DT)

            swp_sb = cp.tile([128, 128], SC_DT, tag="swp")
            idn_sb = cp.tile([128, 128], QKV_DT, tag="idn")
            ons_sb = cp.tile([128, 128], AV_DT, tag="ons")
            gains_sb = cp.tile([128, G], F32, tag="gains")
            msk_sb = cp.tile([128, 4 * QT], AV_DT, tag="msk")
            onsr_sb = cp.tile([128, 128], QKV_DT, tag="onsr")
            nc.gpsimd.memset(onsr_sb[:], 1.0)
            epsq_sb = cp.tile([128, 1], F32, tag="epsq")
            epsk_sb = cp.tile([128, 1], F32, tag="epsk")
            nc.gpsimd.memset(epsq_sb[:], RMS_EPS)
            nc.gpsimd.memset(epsk_sb[:], HD * RMS_EPS)
            warm_sb = cp.tile([128, 1], F32, tag="warm")
            nc.scalar.activation(warm_sb[:], epsq_sb[:], AF.Square)
            nc.scalar.activation(warm_sb[:], epsq_sb[:], AF.Sqrt)
            nc.scalar.activation(warm_sb[:], epsq_sb[:], AF.Exp)
            wp_sb = cp.tile([128, G, D], PR_DT, tag="wp")
            nc.sync.dma_start(out=swp_sb[:], in_=swp[:])
            nc.sync.dma_start(out=idn_sb[:], in_=idn[:])

            # ---------------- Phase A: projections + RMS norm + RoPE -------
            with tc.tile_pool(name="wA", bufs=1) as wa, \
                 tc.tile_pool(name="xs", bufs=6) as xp, \
                 tc.tile_pool(name="sbA", bufs=2) as sa, \
                 tc.tile_pool(name="psA", bufs=1, space="PSUM") as psa, \
                 tc.tile_pool(name="psAm", bufs=2, space="PSUM") as psm:
                wq_sb = wa.tile([128, NCH, G * HD], QKV_DT, tag="wq")
                wk_sb = wa.tile([128, NCH, HD], QKV_DT, tag="wk")
                wv_sb = wa.tile([128, NCH, HD], QKV_DT, tag="wv")
                cos_sb = wa.tile([128, s_len], F32R, tag="cos")
                sin_sb = wa.tile([128, s_len], F32R, tag="sin")
                wqr = wq.rearrange("(c p) m -> p c m", p=128)
                q_ = NCH // 4
                nc.sync.dma_start(out=wq_sb[:, 0:q_, :], in_=wqr[:, 0:q_, :])
                nc.sync.dma_start(out=wk_sb[:, 0:q_, :],
                                  in_=wk.rearrange("(c p) m -> p c m", p=128)[:, 0:q_, :])
                nc.sync.dma_start(out=wv_sb[:, 0:q_, :],
                                  in_=wv.rearrange("(c p) m -> p c m", p=128)[:, 0:q_, :])

                xTr = xT.rearrange("(c p) q -> p c q", p=128)
                for st in range(nst):
                    s0 = st * ST
                    xs_t = xp.tile([128, NCH, ST], QKV_DT, tag="xs", bufs=2)
                    if st == 0:
                        # startup: interleave x chunks with the remaining weight
                        # quarters so the first accumulation round starts ASAP;
                        # rope tables land before s-tile 0 post-processing
                        nc.sync.dma_start(out=xs_t[:, 0:q_, :], in_=xTr[:, 0:q_, s0:s0 + ST])
                        for qq in range(1, 4):
                            sl = slice(qq * q_, (qq + 1) * q_)
                            nc.sync.dma_start(out=wq_sb[:, sl, :], in_=wqr[:, sl, :])
                            nc.sync.dma_start(out=wk_sb[:, sl, :],
                                              in_=wk.rearrange("(c p) m -> p c m", p=128)[:, sl, :])
                            nc.sync.dma_start(out=wv_sb[:, sl, :],
                                              in_=wv.rearrange("(c p) m -> p c m", p=128)[:, sl, :])
                            nc.sync.dma_start(out=xs_t[:, sl, :], in_=xTr[:, sl, s0:s0 + ST])
                            if qq == 1:
                                nc.sync.dma_start(out=cos_sb[:, 0:ST], in_=cos2[:, 0:ST])
                                nc.sync.dma_start(out=sin_sb[:, 0:ST], in_=sin2[:, 0:ST])
                                nc.sync.dma_start(out=gains_sb[:], in_=gains[:])
                    else:
                        nc.sync.dma_start(out=xs_t[:], in_=xTr[:, :, s0:s0 + ST])
                    if st == 1 and nst > 1:
                        nc.sync.dma_start(out=cos_sb[:, ST:s_len], in_=cos2[:, ST:s_len])
                        nc.sync.dma_start(out=sin_sb[:, ST:s_len], in_=sin2[:, ST:s_len])
                        nc.sync.dma_start(out=ons_sb[:], in_=ons[:])
                        nc.sync.dma_start(out=msk_sb[:], in_=msk[:])
                    if st == min(2, nst - 1):
                        if nst == 1:
                            nc.sync.dma_start(out=ons_sb[:], in_=ons[:])
                            nc.sync.dma_start(out=msk_sb[:], in_=msk[:])
                        nc.sync.dma_start(out=wp_sb[:],
                                          in_=wp.rearrange("(h p) n -> p h n", p=128))
                    qp = [psa.tile([128, ST], F32, tag=f"qp{h}", name=f"qp{h}") for h in range(G)]
                    kp = psa.tile([128, ST], F32, tag="kp")
                    vp = psa.tile([128, ST], F32, tag="vp")
                    for c in range(NCH):
                        for h in range(G):
                            nc.tensor.matmul(qp[h][:], wq_sb[:, c, h * HD:(h + 1) * HD],
                                             xs_t[:, c, :], start=(c == 0), stop=(c == NCH - 1))
                        nc.tensor.matmul(kp[:], wk_sb[:, c, :], xs_t[:, c, :],
                                         start=(c == 0), stop=(c == NCH - 1))
                        nc.tensor.matmul(vp[:], wv_sb[:, c, :], xs_t[:, c, :],
                                         start=(c == 0), stop=(c == NCH - 1))

                    # q heads and k: drain + squares first (frees psum banks),
                    # then the rope/norm chains.
                    raws, sqs = {}, {}
                    for t in [G] + list(range(G)):
                        src = qp[t] if t < G else kp
                        raw = sa.tile([128, ST], SC_DT, tag="raw", name=f"raw{t}", bufs=6)
                        nc.vector.tensor_copy(raw[:], src[:])
                        sq = sa.tile([128, ST], QKV_DT, tag="sq", name=f"sq{t}", bufs=6)
                        nc.scalar.activation(sq[:], src[:], AF.Square)
                        raws[t] = raw
                        sqs[t] = sq
                    for t in [G] + list(range(G)):
                        is_q = t < G
                        raw = raws[t]
                        sq = sqs[t]
                        smq = psm.tile([128, ST], F32, tag="misc", name="smq")
                        nc.tensor.matmul(smq[:], onsr_sb[:], sq[:], start=True, stop=True)
                        den = sa.tile([128, ST], F32, tag="den")
                        if is_q:
                            nc.scalar.activation(den[:], smq[:], AF.Sqrt,
                                                 scale=1.0 / HD, bias=epsq_sb[:, 0:1])
                        else:
                            # fold the 1/sqrt(HD) attention scale into k's norm
                            nc.scalar.activation(den[:], smq[:], AF.Sqrt,
                                                 scale=1.0, bias=epsk_sb[:, 0:1])
                        rcp = sa.tile([128, ST], F32, tag="rcp")
                        nc.vector.reciprocal_approx_fast(rcp[:], den[:])
                        qsw = psm.tile([128, ST], F32, tag="misc", name="qsw")
                        nc.tensor.matmul(qsw[:], swp_sb[:], raw[:], start=True, stop=True)
                        m1 = sa.tile([128, ST], F32, tag="m1")
                        nc.vector.tensor_mul(m1[:], raw[:], cos_sb[:, s0:s0 + ST])
                        m2 = sa.tile([128, ST], F32, tag="m2")
                        nc.vector.tensor_mul(m2[:], qsw[:], sin_sb[:, s0:s0 + ST])
                        rope = sa.tile([128, ST], F32, tag="rope")
                        nc.gpsimd.tensor_add(rope[:], m1[:], m2[:])
                        if is_q:
                            nc.vector.scalar_tensor_tensor(
                                out=qhat[:, t, s0:s0 + ST], in0=rope[:],
                                scalar=gains_sb[:, t:t + 1], in1=rcp[:],
                                op0=ALU.mult, op1=ALU.mult)
                        else:
                            nc.gpsimd.tensor_mul(khat[:, s0:s0 + ST], rope[:], rcp[:])

                    # v: drain + PE-transpose into [s_pos, head_dim] chunks
                    vraw = sa.tile([128, ST], QKV_DT, tag="vraw")
                    nc.scalar.copy(vraw[:], vp[:])
                    for j in range(ST // 128):
                        vtp = psm.tile([128, ST], QKV_DT, tag="misc", name="vtp")
                        nc.tensor.transpose(vtp[:, 0:128], vraw[:, j * 128:(j + 1) * 128], idn_sb[:])
                        nc.scalar.copy(vnat[:, (s0 // 128) + j, :], vtp[:, 0:128])

            # ---------------- Phase B+C: attention + output projection -----
            with tc.tile_pool(name="sbB", bufs=3) as sb, \
                 tc.tile_pool(name="sbC", bufs=4) as sc_pool, \
                 tc.tile_pool(name="psY", bufs=1, space="PSUM") as ps_y, \
                 tc.tile_pool(name="psSG", bufs=1, space="PSUM") as ps_sg, \
                 tc.tile_pool(name="psSC", bufs=6, space="PSUM") as ps_sc:

                for i in range(nqt):
                    q0 = i * QT
                    nch_i = (QT // KC) * (i + 1)  # causal: chunks 0..nch_i-1
                    for h in range(G):
                        yp_t = ps_y.tile([128, QT], F32, tag="yp")
                        sgp_t = ps_sg.tile([128, QT], F32, tag="sgp")
                        yp = yp_t[:]
                        sgp = sgp_t[:]
                        ngrp = nch_i // GRP
                        # diagonal (masked) groups first so the exp+mask
                        # latency hides behind the unmasked groups' AV work
                        dg0 = (QT // KC) * i // GRP
                        order = list(range(dg0, ngrp)) + list(range(0, dg0))
                        for gi, g2 in enumerate(order):
                            scp = ps_sc.tile([128, GRP * QT], F32, tag="scp")
                            for j in range(GRP):
                                c = GRP * g2 + j
                                nc.tensor.matmul(scp[:, j * QT:(j + 1) * QT],
                                                 khat[:, c * KC:(c + 1) * KC],
                                                 qhat[:, h, q0:q0 + QT],
                                                 start=True, stop=True)
                            et = sb.tile([128, GRP * QT], AV_DT, tag="et", bufs=5)
                            jd = GRP * g2 - (QT // KC) * i  # diag offset in mask units
                            if gi == 0:
                                # first group of this (h,i): split exp+mask per
                                # chunk so the first AV matmul starts sooner
                                for j in range(GRP):
                                    sl = slice(j * QT, (j + 1) * QT)
                                    nc.scalar.activation(et[:, sl], scp[:, sl], AF.Exp)
                                    if jd + j >= 0:
                                        nc.vector.tensor_mul(
                                            et[:, sl], et[:, sl],
                                            msk_sb[:, (jd + j) * QT:(jd + j + 1) * QT])
                            else:
                                nc.scalar.activation(et[:], scp[:], AF.Exp)
                                if jd + GRP > 0:
                                    nc.vector.tensor_mul(
                                        et[:], et[:],
                                        msk_sb[:, jd * QT:(jd + GRP) * QT])
                            for j in range(GRP):
                                c = GRP * g2 + j
                                nc.tensor.matmul(yp, vnat[:, c, :],
                                                 et[:, j * QT:(j + 1) * QT],
                                                 start=(gi == 0 and j == 0),
                                                 stop=(gi == ngrp - 1 and j == GRP - 1))
                                nc.tensor.matmul(sgp, ons_sb[:],
                                                 et[:, j * QT:(j + 1) * QT],
                                                 start=(gi == 0 and j == 0),
                                                 stop=(gi == ngrp - 1 and j == GRP - 1))
                        rs = sb.tile([128, QT], F32, tag="rs")
                        nc.vector.reciprocal_approx_fast(rs[:], sgp)
                        nc.vector.tensor_mul(yn[:, h, q0:q0 + QT], yp, rs[:])

                    # output projection for this q-tile (all dout chunks)
                    o_acc = sc_pool.tile([128, D // 128, QT], F32, tag="osb", bufs=2)
                    outr = out.rearrange("(dc p) q -> p dc q", p=128)
                    for dc in range(D // 128):
                        op_t = ps_sc.tile([128, GRP * QT], F32, tag="scp", name="op_t")
                        op = op_t[:, 0:QT]
                        for h in range(G):
                            nc.tensor.matmul(op, wp_sb[:, h, dc * 128:(dc + 1) * 128],
                                             yn[:, h, q0:q0 + QT],
                                             start=(h == 0), stop=(h == G - 1))
                        nc.vector.tensor_copy(o_acc[:, dc, :], op)
                        if dc in (7, 11):
                            lo = dc - 7 if dc == 7 else 8
                            nc.sync.dma_start(out=outr[:, lo:dc + 1, q0:q0 + QT],
                                              in_=o_acc[:, lo:dc + 1, :])
                    nc.sync.dma_start(out=outr[:, 12:16, q0:q0 + QT],
                                      in_=o_acc[:, 12:16, :])
    nc.compile()
    return nc


def _host_tables(s_len=S):
    half = HD // 2
    inv_freq = 1.0 / (ROPE_BASE ** (np.arange(0, HD, 2, dtype=np.float64) / HD))
    t = np.arange(s_len, dtype=np.float64)
    freqs = np.outer(inv_freq, t)  # [64, S]
    c = np.cos(freqs)
    s_ = np.sin(freqs)
    cos2 = np.concatenate([c, c], axis=0).astype(np.float32)          # [128, S]
    sin2 = np.concatenate([s_, -s_], axis=0).astype(np.float32)       # [128, S]
    swp = np.zeros((128, 128), dtype=np.float32)
    swp[np.arange(64), np.arange(64) + 64] = 1.0
    swp[np.arange(64) + 64, np.arange(64)] = 1.0
    idn = np.eye(128, dtype=np.float32)
    ons = np.ones((128, 128), dtype=np.float32)
    # causal masks for the 4 diagonal chunk offsets: keep iff 128*j + p <= f
    p = np.arange(128)[:, None]
    f = np.arange(QT)[None, :]
    msk = np.concatenate(
        [((128 * j + p) <= f).astype(np.float32) for j in range(4)], axis=1)
    return cos2, sin2, swp, idn, ons, msk


_NC_CACHE = {}


def _get_program(s_len=S):
    if s_len not in _NC_CACHE:
        _NC_CACHE[s_len] = _build_program(s_len)
    return _NC_CACHE[s_len]


def _np_dt(a, dt_):
    import ml_dtypes
    if dt_ == BF16:
        return np.ascontiguousarray(a).astype(ml_dtypes.bfloat16)
    return np.ascontiguousarray(np.asarray(a, dtype=np.float32))


def make_in_maps(x, Wq, Wk, Wv, Wproj, q_gain, s_len=S):
    x = np.asarray(x, dtype=np.float32)
    Wq = np.asarray(Wq, dtype=np.float32)
    Wk = np.asarray(Wk, dtype=np.float32)
    Wv = np.asarray(Wv, dtype=np.float32)
    Wproj = np.asarray(Wproj, dtype=np.float32)
    q_gain = np.asarray(q_gain, dtype=np.float32)
    cos2, sin2, swp, idn, ons, msk = _host_tables(s_len)
    xT = [np.ascontiguousarray(x[b].T) for b in range(B)]
    in_maps = []
    for core in range(8):
        b, g = core // N_KV, core % N_KV
        in_maps.append({
            "xT": _np_dt(xT[b], QKV_DT),
            "wq": _np_dt(Wq[g * G * HD:(g + 1) * G * HD, :].T, QKV_DT),
            "wk": _np_dt(Wk[g * HD:(g + 1) * HD, :].T, QKV_DT),
            "wv": _np_dt(Wv[g * HD:(g + 1) * HD, :].T, QKV_DT),
            "wp": _np_dt(Wproj[:, g * G * HD:(g + 1) * G * HD].T, PR_DT),
            "cos2": cos2, "sin2": sin2, "swp": _np_dt(swp, SC_DT), "idn": _np_dt(idn, QKV_DT),
            "ons": _np_dt(ons, AV_DT),
            "msk": _np_dt(msk, AV_DT),
            "gains": np.broadcast_to(q_gain[g * G:(g + 1) * G][None, :],
                                     (128, G)).copy(),
        })
    return in_maps


def unshard(results):
    out = np.empty((B, S, D), dtype=np.float32)
    for b in range(B):
        acc = results[4 * b]["out"].astype(np.float32).copy()
        for g in range(1, N_KV):
            acc += results[4 * b + g]["out"]
        out[b] = acc.T
    return out


def kernel(x, Wq, Wk, Wv, Wproj, q_gain):
    nc = _get_program(S)
    in_maps = make_in_maps(x, Wq, Wk, Wv, Wproj, q_gain, S)
    res = run_bass_kernel_spmd(nc, in_maps, list(range(8)))
    return unshard(res.results)



# revision 2
# speedup vs baseline: 1.0366x; 1.0366x over previous
"""Causal GQA self-attention (dense_transformer) on 8 trn2 NeuronCores.

Sharding: core c -> (batch b = c//4, kv-group g = c%4).  Each core computes
the 4 query heads of its kv group against its batch element, plus the
partial output projection for those heads; the host sums the 4 partial
projections per batch (the all-reduce of the tensor-parallel proj).

On-device layout is fully transposed ("feature on partitions"):
  xT [D, S], qhat/khat [head_dim, S], scoresT [k_pos, q_pos], yT [head_dim, S],
  outT [D_out, S].  This makes every matmul contraction land on the partition
  dim with no on-device transposes of activations (RoPE's half-swap is done
  with a permutation matmul, v is transposed head_dim<->seq via the PE).
Softmax is computed without the max subtraction: q/k are RMS-normalized so
|scores| <= gain*sqrt(head_dim) and exp cannot overflow in fp32.

v2 changes vs baseline:
  - all HBM inputs/outputs pre-arranged on host to partition-major
    contiguous layouts (16KB DMA descriptors instead of 256B)
  - PE warmup matmuls at t=0 so the HAM clock gate is at 2.4GHz when the
    first projection matmul issues
  - causal valid-column restriction on scores/exp/AV/sum matmuls
    (the above-diagonal part of the diagonal 512-block-column is skipped)
  - PSUM double-buffering for the attention yp/sgp accumulators
  - bf16 output with 4-way split DMA (host sums partials in f32)
"""

import numpy as np

import concourse.bacc as bacc
import concourse.mybir as mybir
import concourse.tile as tile
from concourse.bass_utils import run_bass_kernel_spmd

F32 = mybir.dt.float32
F32R = mybir.dt.float32r
BF16 = mybir.dt.bfloat16
import os as _os


def _dt_env(name, default="bf16"):
    return BF16 if _os.environ.get(name, default) == "bf16" else F32R


QKV_DT = _dt_env("KERNEL_QKV_DT")   # xs, wq/wk/wv, raw, sq, onsr, vraw/vtp/idn
SC_DT = _dt_env("KERNEL_SC_DT")     # qhat, khat, swp (scores matmul + rope swap)
AV_DT = _dt_env("KERNEL_AV_DT")     # et, vnat, ons, msk (attn-weights path)
PR_DT = _dt_env("KERNEL_PR_DT")     # yn, wp (output projection)
AF = mybir.ActivationFunctionType
ALU = mybir.AluOpType

B = 2
S = 2048
D = 2048
N_HEADS = 16
N_KV = 4
HD = 128
G = N_HEADS // N_KV  # 4 query heads per core
ROPE_BASE = 10000.0
RMS_EPS = 1.1920928955078125e-07
NCH = D // 128       # 16 contraction chunks for the projections
ST = 512             # s-tile width (projection phase)
QT = 512             # q-tile width (attention phase)
KC = 128             # k chunk (scoresT partition block)


def _build_program(s_len=S):
    nst = s_len // ST
    nqt = s_len // QT
    nc = bacc.Bacc("TRN2", target_bir_lowering=False, debug=False, num_devices=8)

    # all pre-arranged: partition dim first, contiguous free dims
    xa = nc.dram_tensor("xa", [128, nst, NCH, ST], QKV_DT, kind="ExternalInput")
    wq = nc.dram_tensor("wq", [128, NCH, G * HD], QKV_DT, kind="ExternalInput")
    wk = nc.dram_tensor("wk", [128, NCH, HD], QKV_DT, kind="ExternalInput")
    wv = nc.dram_tensor("wv", [128, NCH, HD], QKV_DT, kind="ExternalInput")
    wp = nc.dram_tensor("wp", [128, G, D], PR_DT, kind="ExternalInput")
    cos2 = nc.dram_tensor("cos2", [128, s_len], SC_DT, kind="ExternalInput")
    sin2 = nc.dram_tensor("sin2", [128, s_len], SC_DT, kind="ExternalInput")
    gains = nc.dram_tensor("gains", [128, G], F32, kind="ExternalInput")
    swp = nc.dram_tensor("swp", [128, 128], SC_DT, kind="ExternalInput")
    idn = nc.dram_tensor("idn", [128, 128], QKV_DT, kind="ExternalInput")
    ons = nc.dram_tensor("ons", [128, 128], AV_DT, kind="ExternalInput")
    msk = nc.dram_tensor("msk", [128, 128], AV_DT, kind="ExternalInput")
    # out[p, qt, dc, q~] = proj_partial[128*dc + p, 512*qt + q~]
    out = nc.dram_tensor("out", [128, nqt, D // 128, QT], BF16,
                         kind="ExternalOutput")

    with tile.TileContext(nc) as tc:
        with tc.tile_pool(name="persist", bufs=1) as pp, \
             tc.tile_pool(name="const", bufs=1) as cp:
            qhat = pp.tile([128, G, s_len], SC_DT)
            khat = pp.tile([128, s_len], SC_DT)
            vnat = pp.tile([128, s_len // 128, HD], AV_DT)
            yn = pp.tile([128, G, s_len], PR_DT)

            swp_sb = cp.tile([128, 128], SC_DT, tag="swp")
            idn_sb = cp.tile([128, 128], QKV_DT, tag="idn")
            ons_sb = cp.tile([128, 128], AV_DT, tag="ons")
            gains_sb = cp.tile([128, G], F32, tag="gains")
            msk_sb = cp.tile([128, 128], AV_DT, tag="msk")
            onsr_sb = cp.tile([128, 128], QKV_DT, tag="onsr")
            nc.gpsimd.memset(onsr_sb[:], 1.0)
            epsq_sb = cp.tile([128, 1], F32, tag="epsq")
            epsk_sb = cp.tile([128, 1], F32, tag="epsk")
            nc.gpsimd.memset(epsq_sb[:], RMS_EPS)
            nc.gpsimd.memset(epsk_sb[:], HD * RMS_EPS)
            warm_sb = cp.tile([128, 1], F32, tag="warm")
            nc.scalar.activation(warm_sb[:], epsq_sb[:], AF.Square)
            nc.scalar.activation(warm_sb[:], epsq_sb[:], AF.Sqrt)
            nc.scalar.activation(warm_sb[:], epsq_sb[:], AF.Exp)
            wp_sb = cp.tile([128, G, D], PR_DT, tag="wp")
            nc.sync.dma_start(out=swp_sb[:], in_=swp[:])
            nc.sync.dma_start(out=idn_sb[:], in_=idn[:])

            # PE warmup: junk matmuls with no DMA dependency keep the PE
            # busy from t~0.5us so the HAM clock gate reaches 2.4GHz before
            # the first projection matmul and the weight DMAs are hidden.
            with tc.tile_pool(name="wup", bufs=1) as wup, \
                 tc.tile_pool(name="wups", bufs=1, space="PSUM") as wups:
                wu = wup.tile([128, 128], QKV_DT, tag="wu")
                nc.gpsimd.memset(wu[:], 0.0)
                wups_t = wups.tile([128, 128], F32, tag="wups")
                for _ in range(26):
                    nc.tensor.matmul(wups_t[:], wu[:], wu[:],
                                     start=True, stop=True)

            # ---------------- Phase A: projections + RMS norm + RoPE -------
            with tc.tile_pool(name="wA", bufs=1) as wa, \
                 tc.tile_pool(name="xs", bufs=6) as xp, \
                 tc.tile_pool(name="sbA", bufs=2) as sa, \
                 tc.tile_pool(name="psA", bufs=1, space="PSUM") as psa, \
                 tc.tile_pool(name="psAm", bufs=2, space="PSUM") as psm:
                wq_sb = wa.tile([128, NCH, G * HD], QKV_DT, tag="wq")
                wk_sb = wa.tile([128, NCH, HD], QKV_DT, tag="wk")
                wv_sb = wa.tile([128, NCH, HD], QKV_DT, tag="wv")
                cos_sb = wa.tile([128, s_len], SC_DT, tag="cos")
                sin_sb = wa.tile([128, s_len], SC_DT, tag="sin")
                q_ = NCH // 4
                nc.sync.dma_start(out=wq_sb[:, 0:q_, :], in_=wq[:, 0:q_, :])
                nc.sync.dma_start(out=wk_sb[:, 0:q_, :], in_=wk[:, 0:q_, :])
                nc.sync.dma_start(out=wv_sb[:, 0:q_, :], in_=wv[:, 0:q_, :])

                for st in range(nst):
                    s0 = st * ST
                    xs_t = xp.tile([128, NCH, ST], QKV_DT, tag="xs", bufs=2)
                    if st == 0:
                        # startup: interleave x chunks with the remaining weight
                        # quarters so the first accumulation round starts ASAP;
                        # rope tables land before s-tile 0 post-processing
                        nc.sync.dma_start(out=xs_t[:, 0:q_, :], in_=xa[:, 0, 0:q_, :])
                        for qq in range(1, 4):
                            sl = slice(qq * q_, (qq + 1) * q_)
                            nc.sync.dma_start(out=wq_sb[:, sl, :], in_=wq[:, sl, :])
                            nc.sync.dma_start(out=wk_sb[:, sl, :], in_=wk[:, sl, :])
                            nc.sync.dma_start(out=wv_sb[:, sl, :], in_=wv[:, sl, :])
                            nc.sync.dma_start(out=xs_t[:, sl, :], in_=xa[:, 0, sl, :])
                            if qq == 1:
                                nc.sync.dma_start(out=cos_sb[:, 0:ST], in_=cos2[:, 0:ST])
                                nc.sync.dma_start(out=sin_sb[:, 0:ST], in_=sin2[:, 0:ST])
                                nc.sync.dma_start(out=gains_sb[:], in_=gains[:])
                    else:
                        nc.sync.dma_start(out=xs_t[:], in_=xa[:, st, :, :])
                    if st == 1 and nst > 1:
                        nc.sync.dma_start(out=cos_sb[:, ST:s_len], in_=cos2[:, ST:s_len])
                        nc.sync.dma_start(out=sin_sb[:, ST:s_len], in_=sin2[:, ST:s_len])
                        nc.sync.dma_start(out=ons_sb[:], in_=ons[:])
                        nc.sync.dma_start(out=msk_sb[:], in_=msk[:])
                    if st == min(2, nst - 1):
                        if nst == 1:
                            nc.sync.dma_start(out=ons_sb[:], in_=ons[:])
                            nc.sync.dma_start(out=msk_sb[:], in_=msk[:])
                        nc.sync.dma_start(out=wp_sb[:], in_=wp[:])
                    qp = [psa.tile([128, ST], F32, tag=f"qp{h}", name=f"qp{h}") for h in range(G)]
                    kp = psa.tile([128, ST], F32, tag="kp")
                    vp = psa.tile([128, ST], F32, tag="vp")
                    for c in range(NCH):
                        for h in range(G):
                            nc.tensor.matmul(qp[h][:], wq_sb[:, c, h * HD:(h + 1) * HD],
                                             xs_t[:, c, :], start=(c == 0), stop=(c == NCH - 1))
                        nc.tensor.matmul(kp[:], wk_sb[:, c, :], xs_t[:, c, :],
                                         start=(c == 0), stop=(c == NCH - 1))
                        nc.tensor.matmul(vp[:], wv_sb[:, c, :], xs_t[:, c, :],
                                         start=(c == 0), stop=(c == NCH - 1))

                    # q heads and k: drain + squares first (frees psum banks),
                    # then the rope/norm chains.
                    raws, sqs = {}, {}
                    for t in [G] + list(range(G)):
                        src = qp[t] if t < G else kp
                        raw = sa.tile([128, ST], SC_DT, tag="raw", name=f"raw{t}", bufs=6)
                        nc.vector.tensor_copy(raw[:], src[:])
                        sq = sa.tile([128, ST], QKV_DT, tag="sq", name=f"sq{t}", bufs=6)
                        nc.scalar.activation(sq[:], src[:], AF.Square)
                        raws[t] = raw
                        sqs[t] = sq
                    for t in [G] + list(range(G)):
                        is_q = t < G
                        raw = raws[t]
                        sq = sqs[t]
                        smq = psm.tile([128, ST], F32, tag="misc", name="smq")
                        nc.tensor.matmul(smq[:], onsr_sb[:], sq[:], start=True, stop=True)
                        den = sa.tile([128, ST], F32, tag="den")
                        if is_q:
                            nc.scalar.activation(den[:], smq[:], AF.Sqrt,
                                                 scale=1.0 / HD, bias=epsq_sb[:, 0:1])
                        else:
                            # fold the 1/sqrt(HD) attention scale into k's norm
                            nc.scalar.activation(den[:], smq[:], AF.Sqrt,
                                                 scale=1.0, bias=epsk_sb[:, 0:1])
                        rcp = sa.tile([128, ST], F32, tag="rcp")
                        nc.vector.reciprocal_approx_fast(rcp[:], den[:])
                        qsw = psm.tile([128, ST], F32, tag="misc", name="qsw")
                        nc.tensor.matmul(qsw[:], swp_sb[:], raw[:], start=True, stop=True)
                        m1 = sa.tile([128, ST], SC_DT, tag="m1")
                        nc.vector.tensor_mul(m1[:], raw[:], cos_sb[:, s0:s0 + ST])
                        m2 = sa.tile([128, ST], SC_DT, tag="m2")
                        nc.vector.tensor_mul(m2[:], qsw[:], sin_sb[:, s0:s0 + ST])
                        rope = sa.tile([128, ST], SC_DT, tag="rope")
                        nc.gpsimd.tensor_add(rope[:], m1[:], m2[:])
                        if is_q:
                            nc.vector.scalar_tensor_tensor(
                                out=qhat[:, t, s0:s0 + ST], in0=rope[:],
                                scalar=gains_sb[:, t:t + 1], in1=rcp[:],
                                op0=ALU.mult, op1=ALU.mult)
                        else:
                            nc.gpsimd.tensor_mul(khat[:, s0:s0 + ST], rope[:], rcp[:])

                    # v: drain + PE-transpose into [s_pos, head_dim] chunks
                    vraw = sa.tile([128, ST], QKV_DT, tag="vraw")
                    nc.scalar.copy(vraw[:], vp[:])
                    for j in range(ST // 128):
                        vtp = psm.tile([128, ST], QKV_DT, tag="misc", name="vtp")
                        nc.tensor.transpose(vtp[:, 0:128], vraw[:, j * 128:(j + 1) * 128], idn_sb[:])
                        nc.scalar.copy(vnat[:, (s0 // 128) + j, :], vtp[:, 0:128])

            # ---------------- Phase B+C: attention + output projection -----
            with tc.tile_pool(name="sbB", bufs=3) as sb, \
                 tc.tile_pool(name="sbC", bufs=4) as sc_pool, \
                 tc.tile_pool(name="psY", bufs=2, space="PSUM") as ps_y, \
                 tc.tile_pool(name="psSG", bufs=2, space="PSUM") as ps_sg, \
                 tc.tile_pool(name="psSC", bufs=4, space="PSUM") as ps_sc:

                for i in range(nqt):
                    q0 = i * QT
                    nch_i = (QT // KC) * (i + 1)  # causal: chunks 0..nch_i-1
                    for h in range(G):
                        yp_t = ps_y.tile([128, QT], F32, tag="yp")
                        sgp_t = ps_sg.tile([128, QT], F32, tag="sgp")
                        yp = yp_t[:]
                        sgp = sgp_t[:]
                        # diagonal (masked) chunks first so the exp+mask
                        # latency hides behind the unmasked chunks' AV work;
                        # the diagonal chunks only compute their valid
                        # (below-diagonal) column range.
                        dg0 = (QT // KC) * i
                        order = list(range(dg0, nch_i)) + list(range(0, dg0))
                        for gi, c in enumerate(order):
                            off = (c - dg0) * KC if c >= dg0 else 0
                            w = QT - off
                            scp = ps_sc.tile([128, QT], F32, tag="scp")
                            nc.tensor.matmul(scp[:, off:QT],
                                             khat[:, c * KC:(c + 1) * KC],
                                             qhat[:, h, q0 + off:q0 + QT],
                                             start=True, stop=True)
                            et = sb.tile([128, QT], AV_DT, tag="et", bufs=5)
                            nc.scalar.activation(et[:, off:QT], scp[:, off:QT], AF.Exp)
                            if c >= dg0:
                                # triangular mask on the 128-wide diagonal block
                                nc.vector.tensor_mul(
                                    et[:, off:off + KC], et[:, off:off + KC],
                                    msk_sb[:])
                            nc.tensor.matmul(yp[:, off:QT], vnat[:, c, :],
                                             et[:, off:QT],
                                             start=(gi == 0),
                                             stop=(gi == nch_i - 1))
                            nc.tensor.matmul(sgp[:, off:QT], ons_sb[:],
                                             et[:, off:QT],
                                             start=(gi == 0),
                                             stop=(gi == nch_i - 1))
                        rs = sb.tile([128, QT], F32, tag="rs")
                        nc.vector.reciprocal_approx_fast(rs[:], sgp)
                        nc.vector.tensor_mul(yn[:, h, q0:q0 + QT], yp, rs[:])

                    # output projection for this q-tile (all dout chunks)
                    o_acc = sc_pool.tile([128, D // 128, QT], PR_DT, tag="osb", bufs=2)
                    for dc in range(D // 128):
                        op_t = ps_sc.tile([128, QT], F32, tag="scp", name="op_t")
                        op = op_t[:]
                        for h in range(G):
                            nc.tensor.matmul(op, wp_sb[:, h, dc * 128:(dc + 1) * 128],
                                             yn[:, h, q0:q0 + QT],
                                             start=(h == 0), stop=(h == G - 1))
                        nc.vector.tensor_copy(o_acc[:, dc, :], op)
                        if dc in (3, 7, 11, 15):
                            nc.sync.dma_start(out=out[:, i, dc - 3:dc + 1, :],
                                              in_=o_acc[:, dc - 3:dc + 1, :])
    nc.compile()
    return nc


def _host_tables(s_len=S):
    inv_freq = 1.0 / (ROPE_BASE ** (np.arange(0, HD, 2, dtype=np.float64) / HD))
    t = np.arange(s_len, dtype=np.float64)
    freqs = np.outer(inv_freq, t)  # [64, S]
    c = np.cos(freqs)
    s_ = np.sin(freqs)
    cos2 = np.concatenate([c, c], axis=0).astype(np.float32)          # [128, S]
    sin2 = np.concatenate([s_, -s_], axis=0).astype(np.float32)       # [128, S]
    swp = np.zeros((128, 128), dtype=np.float32)
    swp[np.arange(64), np.arange(64) + 64] = 1.0
    swp[np.arange(64) + 64, np.arange(64)] = 1.0
    idn = np.eye(128, dtype=np.float32)
    ons = np.ones((128, 128), dtype=np.float32)
    # strict triangular mask for the 128-wide diagonal blocks: keep iff p <= t
    p = np.arange(128)[:, None]
    f = np.arange(128)[None, :]
    msk = (p <= f).astype(np.float32)
    return cos2, sin2, swp, idn, ons, msk


_NC_CACHE = {}


def _get_program(s_len=S):
    if s_len not in _NC_CACHE:
        _NC_CACHE[s_len] = _build_program(s_len)
    return _NC_CACHE[s_len]


def _np_dt(a, dt_):
    import ml_dtypes
    if dt_ == BF16:
        return np.ascontiguousarray(a).astype(ml_dtypes.bfloat16)
    return np.ascontiguousarray(np.asarray(a, dtype=np.float32))


def make_in_maps(x, Wq, Wk, Wv, Wproj, q_gain, s_len=S):
    x = np.asarray(x, dtype=np.float32)
    Wq = np.asarray(Wq, dtype=np.float32)
    Wk = np.asarray(Wk, dtype=np.float32)
    Wv = np.asarray(Wv, dtype=np.float32)
    Wproj = np.asarray(Wproj, dtype=np.float32)
    q_gain = np.asarray(q_gain, dtype=np.float32)
    cos2, sin2, swp, idn, ons, msk = _host_tables(s_len)
    nst = s_len // ST
    # xa[p, st, c, q~] = x[b].T[128c+p, 512st+q~]
    xas = []
    for b in range(B):
        xT = np.ascontiguousarray(x[b].T)                      # [D, S]
        xa = xT.reshape(NCH, 128, nst, ST).transpose(1, 2, 0, 3)
        xas.append(_np_dt(xa, QKV_DT))
    in_maps = []
    for core in range(8):
        b, g = core // N_KV, core % N_KV
        wqT = np.ascontiguousarray(Wq[g * G * HD:(g + 1) * G * HD, :].T)  # [D, 512]
        wkT = np.ascontiguousarray(Wk[g * HD:(g + 1) * HD, :].T)          # [D, 128]
        wvT = np.ascontiguousarray(Wv[g * HD:(g + 1) * HD, :].T)
        wpT = np.ascontiguousarray(Wproj[:, g * G * HD:(g + 1) * G * HD].T)  # [512, D]
        in_maps.append({
            "xa": xas[b],
            "wq": _np_dt(wqT.reshape(NCH, 128, G * HD).transpose(1, 0, 2), QKV_DT),
            "wk": _np_dt(wkT.reshape(NCH, 128, HD).transpose(1, 0, 2), QKV_DT),
            "wv": _np_dt(wvT.reshape(NCH, 128, HD).transpose(1, 0, 2), QKV_DT),
            "wp": _np_dt(wpT.reshape(G, 128, D).transpose(1, 0, 2), PR_DT),
            "cos2": _np_dt(cos2, SC_DT), "sin2": _np_dt(sin2, SC_DT),
            "swp": _np_dt(swp, SC_DT), "idn": _np_dt(idn, QKV_DT),
            "ons": _np_dt(ons, AV_DT),
            "msk": _np_dt(msk, AV_DT),
            "gains": np.broadcast_to(q_gain[g * G:(g + 1) * G][None, :],
                                     (128, G)).copy(),
        })
    return in_maps


def unshard(results):
    nqt = S // QT
    out = np.empty((B, S, D), dtype=np.float32)
    for b in range(B):
        acc = np.zeros((D, S), dtype=np.float32)
        for g in range(N_KV):
            o = np.asarray(results[4 * b + g]["out"], dtype=np.float32)
            # o[p, qt, dc, q~] -> outT[128dc+p, 512qt+q~]
            acc += o.transpose(2, 0, 1, 3).reshape(D, S)
        out[b] = acc.T
    return out


def kernel(x, Wq, Wk, Wv, Wproj, q_gain):
    nc = _get_program(S)
    in_maps = make_in_maps(x, Wq, Wk, Wv, Wproj, q_gain, S)
    res = run_bass_kernel_spmd(nc, in_maps, list(range(8)))
    return unshard(res.results)


# revision 11
# speedup vs baseline: 1.0504x; 1.0133x over previous
"""Causal GQA self-attention (dense_transformer) on 8 trn2 NeuronCores.

Sharding: core c -> (batch b = c//4, kv-group g = c%4).  Each core computes
the 4 query heads of its kv group against its batch element, plus the
partial output projection for those heads; the host sums the 4 partial
projections per batch (the all-reduce of the tensor-parallel proj).

On-device layout is fully transposed ("feature on partitions"):
  xT [D, S], qhat/khat [head_dim, S], scoresT [k_pos, q_pos], yT [head_dim, S],
  outT [D_out, S].  Softmax runs without max subtraction: q/k are
  RMS-normalized so |scores| <= gain*sqrt(head_dim); exp cannot overflow.

v3 structure (single in-order PE queue, so program order == PE order):
  warmup mms | A(s0) | A(s1)c | B(0)attn | A(s1)post | B(0)proj
             | A(s2)c | B(1)attn | A(s2)post | B(1)proj
             | A(s3)c | B(2)attn | A(s3)post | B(2)proj | B(3)
  Each A(s)post's matmuls depend on ACT/DVE drains of the c-loop; the
  interleaved B segment fills the PE while those drains run, which also
  keeps the HAM clock gate at 2.4GHz throughout.  PSUM is one shared set
  of 8 banks: qp0-3/kp/vp (A accumulators, reused as scores/yp/op tiles
  in B via pool-tag sharing + WAR tracking) and 2 rotating misc banks.
  DMA is split over the two HWDGE queues: weights/tables on the ACT
  queue, x/out on the SP queue, with chunk-granular gating at startup.
"""

import numpy as np

import concourse.bacc as bacc
import concourse.mybir as mybir
import concourse.tile as tile
from concourse.bass_utils import run_bass_kernel_spmd

F32 = mybir.dt.float32
F32R = mybir.dt.float32r
BF16 = mybir.dt.bfloat16
import os as _os


def _dt_env(name, default="bf16"):
    return BF16 if _os.environ.get(name, default) == "bf16" else F32R


QKV_DT = _dt_env("KERNEL_QKV_DT")
SC_DT = _dt_env("KERNEL_SC_DT")
AV_DT = _dt_env("KERNEL_AV_DT")
PR_DT = _dt_env("KERNEL_PR_DT")
AF = mybir.ActivationFunctionType
ALU = mybir.AluOpType

B = 2
S = 2048
D = 2048
N_HEADS = 16
N_KV = 4
HD = 128
G = N_HEADS // N_KV
ROPE_BASE = 10000.0
RMS_EPS = 1.1920928955078125e-07
NCH = D // 128
ST = 512
QT = 512
KC = 128


def _build_program(s_len=S):
    nst = s_len // ST
    nqt = s_len // QT
    nc = bacc.Bacc("TRN2", target_bir_lowering=False, debug=False, num_devices=8)

    xa = nc.dram_tensor("xa", [128, nst, NCH, ST], QKV_DT, kind="ExternalInput")
    wq = nc.dram_tensor("wq", [128, NCH, G * HD], QKV_DT, kind="ExternalInput")
    wk = nc.dram_tensor("wk", [128, NCH, HD], QKV_DT, kind="ExternalInput")
    wv = nc.dram_tensor("wv", [128, NCH, HD], QKV_DT, kind="ExternalInput")
    wp = nc.dram_tensor("wp", [128, G, D], PR_DT, kind="ExternalInput")
    cos2 = nc.dram_tensor("cos2", [128, s_len], SC_DT, kind="ExternalInput")
    sin2 = nc.dram_tensor("sin2", [128, s_len], SC_DT, kind="ExternalInput")
    gains = nc.dram_tensor("gains", [128, G], F32, kind="ExternalInput")
    swp = nc.dram_tensor("swp", [128, 128], SC_DT, kind="ExternalInput")
    idn = nc.dram_tensor("idn", [128, 128], QKV_DT, kind="ExternalInput")
    ons = nc.dram_tensor("ons", [128, 128], AV_DT, kind="ExternalInput")
    msk = nc.dram_tensor("msk", [128, 128], AV_DT, kind="ExternalInput")
    # out[p, qt, dc, q~] = proj_partial[128*dc + p, 512*qt + q~]
    out = nc.dram_tensor("out", [128, nqt, D // 128, QT], BF16,
                         kind="ExternalOutput")

    with tile.TileContext(nc) as tc:
        with tc.tile_pool(name="persist", bufs=1) as pp, \
             tc.tile_pool(name="const", bufs=1) as cp, \
             tc.tile_pool(name="wA", bufs=1) as wa, \
             tc.tile_pool(name="xs", bufs=2) as xp, \
             tc.tile_pool(name="sbA", bufs=2) as sa, \
             tc.tile_pool(name="sbB", bufs=3) as sb, \
             tc.tile_pool(name="sbC", bufs=2) as sc_pool, \
             tc.tile_pool(name="psA", bufs=1, space="PSUM") as psa, \
             tc.tile_pool(name="psAm", bufs=2, space="PSUM") as psm:
            qhat = pp.tile([128, G, s_len], SC_DT)
            khat = pp.tile([128, s_len], SC_DT)
            vnat = pp.tile([128, s_len // 128, HD], AV_DT)
            yn = pp.tile([128, G, s_len], PR_DT)

            swp_sb = cp.tile([128, 128], SC_DT, tag="swp")
            idn_sb = cp.tile([128, 128], QKV_DT, tag="idn")
            ons_sb = cp.tile([128, 128], AV_DT, tag="ons")
            gains_sb = cp.tile([128, G], F32, tag="gains")
            msk_sb = cp.tile([128, 128], AV_DT, tag="msk")
            onsr_sb = cp.tile([128, 128], QKV_DT, tag="onsr")
            nc.gpsimd.memset(onsr_sb[:], 1.0)
            epsq_sb = cp.tile([128, 1], F32, tag="epsq")
            epsk_sb = cp.tile([128, 1], F32, tag="epsk")
            nc.gpsimd.memset(epsq_sb[:], RMS_EPS)
            nc.gpsimd.memset(epsk_sb[:], HD * RMS_EPS)
            warm_sb = cp.tile([128, 1], F32, tag="warm")
            nc.scalar.activation(warm_sb[:], epsq_sb[:], AF.Square)
            nc.scalar.activation(warm_sb[:], epsq_sb[:], AF.Sqrt)
            nc.scalar.activation(warm_sb[:], epsq_sb[:], AF.Exp)
            wp_sb = cp.tile([128, G, D], PR_DT, tag="wp")
            nc.scalar.dma_start(out=swp_sb[:], in_=swp[:])
            nc.scalar.dma_start(out=idn_sb[:], in_=idn[:])

            wq_sb = wa.tile([128, NCH, G * HD], QKV_DT, tag="wq")
            wk_sb = wa.tile([128, NCH, HD], QKV_DT, tag="wk")
            wv_sb = wa.tile([128, NCH, HD], QKV_DT, tag="wv")
            cos_sb = wa.tile([128, s_len], SC_DT, tag="cos")
            sin_sb = wa.tile([128, s_len], SC_DT, tag="sin")

            # PE warmup: junk matmuls (no DMA dependency) keep the PE busy
            # through the startup DMA so the HAM clock gate reaches 2.4GHz
            # before the first projection matmul.
            wu = cp.tile([128, 256], QKV_DT, tag="wu")
            nc.gpsimd.memset(wu[:], 0.0)
            wu_ps = psm.tile([128, ST], F32, tag="misc", name="wups")
            for _ in range(18):
                nc.tensor.matmul(wu_ps[:, 0:256], wu[:, 0:128], wu[:],
                                 start=True, stop=True)

            # ------------- emit helpers -------------
            def a_cloop(st):
                s0 = st * ST
                xs_t = xp.tile([128, NCH, ST], QKV_DT, tag="xs")
                if st == 0:
                    # chunk-granular startup: first matmul only needs chunk 0
                    for c4 in range(4):
                        nc.sync.dma_start(out=xs_t[:, c4, :], in_=xa[:, 0, c4, :])
                        if c4 == 0:
                            nc.scalar.dma_start(out=wq_sb[:, 0:1, :], in_=wq[:, 0:1, :])
                            nc.scalar.dma_start(out=wk_sb[:, 0:1, :], in_=wk[:, 0:1, :])
                            nc.scalar.dma_start(out=wv_sb[:, 0:1, :], in_=wv[:, 0:1, :])
                        elif c4 == 1:
                            nc.scalar.dma_start(out=wq_sb[:, 1:4, :], in_=wq[:, 1:4, :])
                            nc.scalar.dma_start(out=wk_sb[:, 1:4, :], in_=wk[:, 1:4, :])
                            nc.scalar.dma_start(out=wv_sb[:, 1:4, :], in_=wv[:, 1:4, :])
                    nc.sync.dma_start(out=xs_t[:, 4:8, :], in_=xa[:, 0, 4:8, :])
                    for qq in range(1, 4):
                        sl = slice(qq * 4, (qq + 1) * 4)
                        nc.scalar.dma_start(out=wq_sb[:, sl, :], in_=wq[:, sl, :])
                        nc.scalar.dma_start(out=wk_sb[:, sl, :], in_=wk[:, sl, :])
                        nc.scalar.dma_start(out=wv_sb[:, sl, :], in_=wv[:, sl, :])
                        if qq < 3:
                            sl2 = slice((qq + 1) * 4, (qq + 2) * 4)
                            nc.sync.dma_start(out=xs_t[:, sl2, :], in_=xa[:, 0, sl2, :])
                        if qq == 1:
                            nc.scalar.dma_start(out=cos_sb[:, 0:ST], in_=cos2[:, 0:ST])
                            nc.scalar.dma_start(out=sin_sb[:, 0:ST], in_=sin2[:, 0:ST])
                            nc.scalar.dma_start(out=gains_sb[:], in_=gains[:])
                else:
                    nc.sync.dma_start(out=xs_t[:], in_=xa[:, st, :, :])
                if st == 1:
                    nc.scalar.dma_start(out=cos_sb[:, ST:s_len], in_=cos2[:, ST:s_len])
                    nc.scalar.dma_start(out=sin_sb[:, ST:s_len], in_=sin2[:, ST:s_len])
                    nc.scalar.dma_start(out=ons_sb[:], in_=ons[:])
                    nc.scalar.dma_start(out=msk_sb[:], in_=msk[:])
                    nc.scalar.dma_start(out=wp_sb[:], in_=wp[:])
                qp = [psa.tile([128, ST], F32, tag=f"qp{h}", name=f"qp{h}") for h in range(G)]
                kp = psa.tile([128, ST], F32, tag="kp")
                vp = psa.tile([128, ST], F32, tag="vp")
                for c in range(NCH):
                    for h in range(G):
                        nc.tensor.matmul(qp[h][:], wq_sb[:, c, h * HD:(h + 1) * HD],
                                         xs_t[:, c, :], start=(c == 0), stop=(c == NCH - 1))
                    nc.tensor.matmul(kp[:], wk_sb[:, c, :], xs_t[:, c, :],
                                     start=(c == 0), stop=(c == NCH - 1))
                    nc.tensor.matmul(vp[:], wv_sb[:, c, :], xs_t[:, c, :],
                                     start=(c == 0), stop=(c == NCH - 1))
                return qp, kp, vp

            def a_drain(st, qp, kp, vp):
                """Drain the A psum accumulators to SBUF (DVE copies only),
                ordered by what the following b_attn segment unblocks first.
                Emitted BEFORE b_attn so the shared psum banks hand off
                without a cross-engine dependency cycle."""
                raws = {}
                for t in [G, 0, 1, "v", 2, 3]:
                    if t == "v":
                        raw = sa.tile([128, ST], QKV_DT, tag="vraw")
                        nc.vector.tensor_copy(raw[:], vp[:])
                    else:
                        src = qp[t] if t < G else kp
                        raw = sa.tile([128, ST], SC_DT, tag="raw", name=f"raw{t}", bufs=6)
                        nc.vector.tensor_copy(raw[:], src[:])
                    raws[t] = raw
                return raws

            def _chain_one(st, t, raw):
                s0 = st * ST
                is_q = t < G
                sq = sa.tile([128, ST], QKV_DT, tag="sq", name=f"sq{t}", bufs=2)
                nc.scalar.activation(sq[:], raw[:], AF.Square)
                smq = psm.tile([128, ST], F32, tag="misc", name="smq")
                nc.tensor.matmul(smq[:], onsr_sb[:], sq[:], start=True, stop=True)
                den = sa.tile([128, ST], F32, tag="den")
                if is_q:
                    nc.scalar.activation(den[:], smq[:], AF.Sqrt,
                                         scale=1.0 / HD, bias=epsq_sb[:, 0:1])
                else:
                    nc.scalar.activation(den[:], smq[:], AF.Sqrt,
                                         scale=1.0, bias=epsk_sb[:, 0:1])
                rcp = sa.tile([128, ST], F32, tag="rcp")
                nc.vector.reciprocal_approx_fast(rcp[:], den[:])
                qsw = psm.tile([128, ST], F32, tag="misc", name="qsw")
                nc.tensor.matmul(qsw[:], swp_sb[:], raw[:], start=True, stop=True)
                m1 = sa.tile([128, ST], SC_DT, tag="m1")
                nc.vector.tensor_mul(m1[:], raw[:], cos_sb[:, s0:s0 + ST])
                m2 = sa.tile([128, ST], SC_DT, tag="m2")
                nc.vector.tensor_mul(m2[:], qsw[:], sin_sb[:, s0:s0 + ST])
                rope = sa.tile([128, ST], SC_DT, tag="rope")
                nc.gpsimd.tensor_add(rope[:], m1[:], m2[:])
                if is_q:
                    nc.vector.scalar_tensor_tensor(
                        out=qhat[:, t, s0:s0 + ST], in0=rope[:],
                        scalar=gains_sb[:, t:t + 1], in1=rcp[:],
                        op0=ALU.mult, op1=ALU.mult)
                else:
                    nc.gpsimd.tensor_mul(khat[:, s0:s0 + ST], rope[:], rcp[:])

            def a_chains_k(st, raws):
                s0 = st * ST
                _chain_one(st, G, raws[G])
                vraw = raws["v"]
                for j in range(ST // 128):
                    vtp = psm.tile([128, ST], QKV_DT, tag="misc", name="vtp")
                    nc.tensor.transpose(vtp[:, 0:128], vraw[:, j * 128:(j + 1) * 128], idn_sb[:])
                    nc.scalar.copy(vnat[:, (s0 // 128) + j, :], vtp[:, 0:128])

            def a_chains_q(st, raws):
                for t in range(G):
                    _chain_one(st, t, raws[t])

            def b_attn(i):
                q0 = i * QT
                nch_i = (QT // KC) * (i + 1)
                for h in range(G):
                    yp_t = psa.tile([128, QT], F32, tag=("kp" if h % 2 == 0 else "vp"),
                                    name=f"yp{h}")
                    sgp_t = psm.tile([128, QT], F32, tag="misc", name=f"sgp{h}")
                    yp = yp_t[:]
                    sgp = sgp_t[:]
                    dg0 = (QT // KC) * i
                    order = list(range(dg0, nch_i)) + list(range(0, dg0))
                    for gi, c in enumerate(order):
                        off = (c - dg0) * KC if c >= dg0 else 0
                        # gi=0 borrows a misc bank (free at segment entry) so
                        # the first scores matmul never waits on the A drains
                        if gi == 0:
                            scp = psm.tile([128, QT], F32, tag="misc", name="scp0")
                        else:
                            scp = psa.tile([128, QT], F32, tag=f"qp{(gi - 1) % 4}",
                                           name="scp")
                        nc.tensor.matmul(scp[:, off:QT],
                                         khat[:, c * KC:(c + 1) * KC],
                                         qhat[:, h, q0 + off:q0 + QT],
                                         start=True, stop=True)
                        et = sb.tile([128, QT], AV_DT, tag="et", bufs=5)
                        nc.scalar.activation(et[:, off:QT], scp[:, off:QT], AF.Exp)
                        if c >= dg0:
                            # triangular mask on the diagonal 128-block; on
                            # gpsimd (idle here) so it never queues behind the
                            # DVE drain backlog at segment entry
                            nc.gpsimd.tensor_mul(
                                et[:, off:off + KC], et[:, off:off + KC],
                                msk_sb[:])
                        nc.tensor.matmul(yp[:, off:QT], vnat[:, c, :],
                                         et[:, off:QT],
                                         start=(gi == 0),
                                         stop=(gi == nch_i - 1))
                        nc.tensor.matmul(sgp[:, off:QT], ons_sb[:],
                                         et[:, off:QT],
                                         start=(gi == 0),
                                         stop=(gi == nch_i - 1))
                    rs = sb.tile([128, QT], F32, tag="rs")
                    nc.vector.reciprocal_approx_fast(rs[:], sgp)
                    nc.vector.tensor_mul(yn[:, h, q0:q0 + QT], yp, rs[:])

            def b_proj(i):
                q0 = i * QT
                o_acc = sc_pool.tile([128, D // 128, QT], PR_DT, tag="osb")
                last = (i == nqt - 1)
                for dc in range(D // 128):
                    op_t = psa.tile([128, QT], F32, tag=f"qp{dc % 4}", name="op_t")
                    op = op_t[:]
                    for h in range(G):
                        nc.tensor.matmul(op, wp_sb[:, h, dc * 128:(dc + 1) * 128],
                                         yn[:, h, q0:q0 + QT],
                                         start=(h == 0), stop=(h == G - 1))
                    nc.vector.tensor_copy(o_acc[:, dc, :], op)
                    # out rides the ACT HWDGE queue so a gated out descriptor
                    # never head-blocks the x loads on the SP queue
                    if last:
                        # fine-grained tail: ship each dout chunk as it lands
                        nc.scalar.dma_start(out=out[:, i, dc:dc + 1, :],
                                            in_=o_acc[:, dc:dc + 1, :])
                    elif dc in (3, 7, 11, 15):
                        nc.scalar.dma_start(out=out[:, i, dc - 3:dc + 1, :],
                                            in_=o_acc[:, dc - 3:dc + 1, :])

            # ------------- emission order -------------
            qkv = a_cloop(0)
            rr = a_drain(0, *qkv)
            a_chains_k(0, rr)
            a_chains_q(0, rr)
            for st in range(1, nst):
                qkv = a_cloop(st)
                rr = a_drain(st, *qkv)
                b_attn(st - 1)
                a_chains_k(st, rr)
                b_proj(st - 1)
                a_chains_q(st, rr)
            b_attn(nqt - 1)
            b_proj(nqt - 1)
    nc.compile()
    return nc


def _host_tables(s_len=S):
    inv_freq = 1.0 / (ROPE_BASE ** (np.arange(0, HD, 2, dtype=np.float64) / HD))
    t = np.arange(s_len, dtype=np.float64)
    freqs = np.outer(inv_freq, t)  # [64, S]
    c = np.cos(freqs)
    s_ = np.sin(freqs)
    cos2 = np.concatenate([c, c], axis=0).astype(np.float32)
    sin2 = np.concatenate([s_, -s_], axis=0).astype(np.float32)
    swp = np.zeros((128, 128), dtype=np.float32)
    swp[np.arange(64), np.arange(64) + 64] = 1.0
    swp[np.arange(64) + 64, np.arange(64)] = 1.0
    idn = np.eye(128, dtype=np.float32)
    ons = np.ones((128, 128), dtype=np.float32)
    p = np.arange(128)[:, None]
    f = np.arange(128)[None, :]
    msk = (p <= f).astype(np.float32)
    return cos2, sin2, swp, idn, ons, msk


_NC_CACHE = {}


def _get_program(s_len=S):
    if s_len not in _NC_CACHE:
        _NC_CACHE[s_len] = _build_program(s_len)
    return _NC_CACHE[s_len]


def _np_dt(a, dt_):
    import ml_dtypes
    if dt_ == BF16:
        return np.ascontiguousarray(a).astype(ml_dtypes.bfloat16)
    return np.ascontiguousarray(np.asarray(a, dtype=np.float32))


def make_in_maps(x, Wq, Wk, Wv, Wproj, q_gain, s_len=S):
    x = np.asarray(x, dtype=np.float32)
    Wq = np.asarray(Wq, dtype=np.float32)
    Wk = np.asarray(Wk, dtype=np.float32)
    Wv = np.asarray(Wv, dtype=np.float32)
    Wproj = np.asarray(Wproj, dtype=np.float32)
    q_gain = np.asarray(q_gain, dtype=np.float32)
    cos2, sin2, swp, idn, ons, msk = _host_tables(s_len)
    nst = s_len // ST
    xas = []
    for b in range(B):
        xT = np.ascontiguousarray(x[b].T)                      # [D, S]
        xa = xT.reshape(NCH, 128, nst, ST).transpose(1, 2, 0, 3)
        xas.append(_np_dt(xa, QKV_DT))
    in_maps = []
    for core in range(8):
        b, g = core // N_KV, core % N_KV
        wqT = np.ascontiguousarray(Wq[g * G * HD:(g + 1) * G * HD, :].T)
        wkT = np.ascontiguousarray(Wk[g * HD:(g + 1) * HD, :].T)
        wvT = np.ascontiguousarray(Wv[g * HD:(g + 1) * HD, :].T)
        wpT = np.ascontiguousarray(Wproj[:, g * G * HD:(g + 1) * G * HD].T)
        in_maps.append({
            "xa": xas[b],
            "wq": _np_dt(wqT.reshape(NCH, 128, G * HD).transpose(1, 0, 2), QKV_DT),
            "wk": _np_dt(wkT.reshape(NCH, 128, HD).transpose(1, 0, 2), QKV_DT),
            "wv": _np_dt(wvT.reshape(NCH, 128, HD).transpose(1, 0, 2), QKV_DT),
            "wp": _np_dt(wpT.reshape(G, 128, D).transpose(1, 0, 2), PR_DT),
            "cos2": _np_dt(cos2, SC_DT), "sin2": _np_dt(sin2, SC_DT),
            "swp": _np_dt(swp, SC_DT), "idn": _np_dt(idn, QKV_DT),
            "ons": _np_dt(ons, AV_DT),
            "msk": _np_dt(msk, AV_DT),
            "gains": np.broadcast_to(q_gain[g * G:(g + 1) * G][None, :],
                                     (128, G)).copy(),
        })
    return in_maps


def unshard(results):
    out = np.empty((B, S, D), dtype=np.float32)
    for b in range(B):
        acc = np.zeros((D, S), dtype=np.float32)
        for g in range(N_KV):
            o = np.asarray(results[4 * b + g]["out"], dtype=np.float32)
            acc += o.transpose(2, 0, 1, 3).reshape(D, S)
        out[b] = acc.T
    return out


def kernel(x, Wq, Wk, Wv, Wproj, q_gain):
    nc = _get_program(S)
    in_maps = make_in_maps(x, Wq, Wk, Wv, Wproj, q_gain, S)
    res = run_bass_kernel_spmd(nc, in_maps, list(range(8)))
    return unshard(res.results)
